# revision 1
# baseline (speedup 1.0000x reference)
"""Causal MQA kernel for Trainium2, SPMD over 8 NeuronCores.

Sharding: tensor-parallel over query heads (16 heads / 8 cores = 2 heads per
core); the single shared KV head is replicated (classic MQA TP layout). Each
core computes the full kv projection, its 2 query heads' projection, causal
attention for those heads, and writes its [B, T, 256] slice of the output.
The host concatenates slices along the channel dim (no device collectives).

Device algorithm (per core, per batch):
  - x arrives transposed (xT[b] = x[b].T, [C, T]) so the projections emit
    kT/vT/qT in [head_dim, T] layout directly.
  - S^T[k, q] = matmul(lhsT=kT_blk, rhs=qT_chunk): keys on partitions.
  - P^T = exp(S^T / sqrt(hd)) with no max-subtraction (scores are O(1) for
    this problem's 0.02-scaled weights, exp cannot overflow); causal mask
    applied multiplicatively after exp on diagonal blocks only.
  - y^T [d, q] accumulates in PSUM via matmul(lhsT=V_blk [keys, d], rhs=P^T);
    softmax denominators accumulate via matmul(lhsT=ones [keys, 1]).
  - y = (y^T * broadcast(1/sums)).T via PE transpose, DMA'd out.
All matmuls run as float32r (full-rate fp32 mode on the PE array at N=512).
"""

import math
from contextlib import ExitStack

import numpy as np

import concourse.bass as bass
import concourse.mybir as mybir
import concourse.tile as tile
from concourse import bacc
from concourse.bass_utils import run_bass_kernel_spmd
from concourse.masks import make_identity

F32 = mybir.dt.float32
F32R = mybir.dt.float32r
P = 128  # partitions
HD = 128  # head dim
QC = 512  # query-chunk width (one fp32 PSUM bank)
KGRP = 2  # key tiles per exp group
N_CORES = 8


def r(ap):
    return ap.bitcast(F32R)


PHASE_MARKS = []


def _mark(nc, name):
    n = int(nc.get_next_instruction_name().split("-")[-1])
    PHASE_MARKS.append((n, name))


def build_nc(B, T, C, HPC):
    """Build the per-core Bass program. HPC = query heads per core.

    Pipeline: per batch, T is processed in quarters of QC=512 query rows.
    Each quarter loads its x^T column block (all C rows), projects that
    block's kT/vT/qT, transposes v, and runs causal attention for its
    query chunk (which only needs kT/v of quarters <= this one). This
    interleaves projection matmuls with attention tails so the PE never
    waits on DMA after the first chunk.
    """
    NQC = T // QC  # query chunks == T-quarters
    NCC = C // P  # contraction chunks
    KTQ = QC // P  # key tiles per query chunk (4)
    inv_sqrt_hd = 1.0 / math.sqrt(HD)

    nc = bacc.Bacc("TRN2", target_bir_lowering=False, debug=False,
                   num_devices=N_CORES)
    xT = nc.dram_tensor("xT", [B, C, T], F32, kind="ExternalInput").ap()
    wq_t = nc.dram_tensor("wq_t", [C, HPC * HD], F32, kind="ExternalInput").ap()
    wkv_t = nc.dram_tensor("wkv_t", [C, 2 * HD], F32, kind="ExternalInput").ap()
    y = nc.dram_tensor("y", [B, T, HPC * HD], F32, kind="ExternalOutput").ap()

    with tile.TileContext(nc) as tc, ExitStack() as ctx, \
            nc.allow_low_precision(reason="float32r tiles (~19-bit mantissa) feed the PE; accumulation stays fp32 in PSUM"):
        consts = ctx.enter_context(tc.tile_pool(name="consts", bufs=1))
        identity = consts.tile([P, P], F32)
        make_identity(nc, identity)
        ones_f32 = consts.tile([P, 1], F32)
        nc.gpsimd.memset(ones_f32, 1.0)
        ones_col = consts.tile([P, 1], F32R)
        nc.vector.tensor_copy(ones_col, ones_f32)
        ones_rf32 = consts.tile([1, P], F32)
        nc.gpsimd.memset(ones_rf32, 1.0)
        ones_row = consts.tile([1, P], F32R)
        nc.vector.tensor_copy(ones_row, ones_rf32)

        # Causal masks for the two diagonal key-tile groups of each query
        # chunk. mask[k, u, q] = 1 iff q >= k + 128*u + off  (off = 0, 256).
        masks = []
        for off in (0, KGRP * P):
            m = consts.tile([P, KGRP, QC], F32, tag=f"mask{off}")
            nc.gpsimd.memset(m, 1.0)
            nc.gpsimd.affine_select(
                out=m, in_=m,
                pattern=[[-P, KGRP], [1, QC]],
                compare_op=mybir.AluOpType.is_ge,
                fill=0.0,
                base=-off,
                channel_multiplier=-1,
            )
            masks.append(m)

        # kv weights first (kT/vT projections consume them immediately);
        # q weights queued behind the first x tiles.
        wkv_sb = consts.tile([P, NCC, 2 * HD], F32R, tag="wkv")
        wkv_r = wkv_t.rearrange("(cc p) d -> p cc d", p=P)
        for c0 in range(0, NCC, 4):
            nc.sync.dma_start(out=wkv_sb[:, c0:c0 + 4],
                              in_=r(wkv_r[:, c0:c0 + 4]))
        wq_sb = consts.tile([P, NCC, HPC * HD], F32R, tag="wq")

        xt_pool = ctx.enter_context(tc.tile_pool(name="xt", bufs=NCC + 8))
        kT_pool = ctx.enter_context(tc.tile_pool(name="kT", bufs=2))
        v_pool = ctx.enter_context(tc.tile_pool(name="v", bufs=2))
        vT_pool = ctx.enter_context(tc.tile_pool(name="vT", bufs=2))
        qT_pool = ctx.enter_context(tc.tile_pool(name="qT", bufs=2))
        pt_pool = ctx.enter_context(tc.tile_pool(name="pt", bufs=4))
        ysum_pool = ctx.enter_context(tc.tile_pool(name="ysum", bufs=4))
        sums_sb_pool = ctx.enter_context(tc.tile_pool(name="ssb", bufs=3))
        yout_pool = ctx.enter_context(tc.tile_pool(name="yout", bufs=2))
        recip_pool = ctx.enter_context(tc.tile_pool(name="recip", bufs=3))

        # PSUM budget (8 banks): st 2x2 + y 2 + sums 1 + ytr/bc/vtr 1.
        # Projection accumulators share st's slots; v-transposes share ytr's.
        st_pp = ctx.enter_context(tc.tile_pool(name="st_pp", bufs=2,
                                               space="PSUM"))
        y_pp = ctx.enter_context(tc.tile_pool(name="y_pp", bufs=2,
                                              space="PSUM"))
        sums_pp = ctx.enter_context(tc.tile_pool(name="sums_pp", bufs=1,
                                                 space="PSUM"))
        ytr_pp = ctx.enter_context(tc.tile_pool(name="ytr_pp", bufs=1,
                                                space="PSUM"))

        pending_tails = []

        def emit_tail(tb, tqc, ysums, sums2, terminal=False):
            # Deferred: transpose both heads' sums to [128, nqt*HPC] (tiny PE
            # transposes, both heads per instruction) so the reciprocal runs
            # across all DVE lanes (a [1,512] reciprocal is single-lane and
            # costs ~3.3us), then fold the softmax normalization into
            # per-partition ACT scales on the final PSUM->SBUF copies of the
            # transposed output.
            with nc.named_scope(f"ltail{tb}q{tqc}"):
                _mark(nc, f"b{tb}q{tqc}:ltail")
                pp = st_pp if terminal else ytr_pp
                tg = "st" if terminal else "ytr"
                NS2 = (HPC - 1) * 32 + 1
                rt_ps = pp.tile([P, KTQ, NS2], F32, tag=tg)
                for qt in range(KTQ):
                    nc.tensor.transpose(rt_ps[:, qt],
                                        sums2[:, qt * P:(qt + 1) * P],
                                        identity[0:NS2, 0:NS2])
                rt = recip_pool.tile([P, KTQ, NS2], F32, tag="recip")
                nc.vector.reciprocal(rt, rt_ps)
                for th in range(HPC):
                    ytr = pp.tile([P, QC], F32, tag=tg)
                    for qt in range(KTQ):
                        nc.tensor.transpose(ytr[:, qt * P:(qt + 1) * P],
                                            ysums[th][:, qt * P:(qt + 1) * P],
                                            identity)
                    yo = yout_pool.tile([P, QC], F32, tag="yo")
                    for qt in range(KTQ):
                        nc.scalar.activation(
                            yo[:, qt * P:(qt + 1) * P],
                            ytr[:, qt * P:(qt + 1) * P],
                            mybir.ActivationFunctionType.Copy,
                            scale=rt[:, qt, th * 32:th * 32 + 1])
                    ydst = y[tb].rearrange(
                        "(nq qt p) (h d) -> nq h p qt d",
                        qt=KTQ, p=P, h=HPC)[tqc, th]
                    nc.sync.dma_start(
                        out=ydst,
                        in_=yo.rearrange("p (qt d) -> p qt d", qt=KTQ))

        wq_loaded = False
        for b in range(B):
            kT = kT_pool.tile([P, T], F32R, tag="kT")
            v_sb = v_pool.tile([P, T], F32R, tag="v")

            for tq in range(NQC):
                _mark(nc, f"b{b}q{tq}")
                tslc = slice(tq * QC, (tq + 1) * QC)
                with nc.named_scope(f"load{b}q{tq}"):
                    xts = []
                    for cc in range(NCC):
                        xtile = xt_pool.tile([P, QC], F32R, tag="xt")
                        nc.sync.dma_start(
                            out=xtile, in_=r(xT[b, cc * P:(cc + 1) * P, tslc]))
                        xts.append(xtile)
                    if not wq_loaded:
                        wq_r = wq_t.rearrange("(cc p) d -> p cc d", p=P)
                        for c0 in range(0, NCC, 4):
                            nc.sync.dma_start(out=wq_sb[:, c0:c0 + 4],
                                              in_=r(wq_r[:, c0:c0 + 4]))
                        wq_loaded = True

                # ---- projections for this T-quarter ----
                with nc.named_scope(f"proj{b}q{tq}"):
                    vTq = vT_pool.tile([P, QC], F32, tag="vT")
                    qTq = qT_pool.tile([P, HPC, QC], F32R, tag="qT")
                    outs = [(kT[:, tslc], wkv_sb, 0), (vTq, wkv_sb, 1)]
                    outs += [(qTq[:, h], wq_sb, h) for h in range(HPC)]
                    for oi, (dst, wsb, m) in enumerate(outs):
                        _mark(nc, f"b{b}q{tq}:proj{oi}")
                        ps = st_pp.tile([P, QC], F32, tag="st")
                        for cc in range(NCC):
                            nc.tensor.matmul(
                                ps,
                                lhsT=wsb[:, cc, m * HD:(m + 1) * HD],
                                rhs=xts[cc],
                                start=(cc == 0), stop=(cc == NCC - 1),
                            )
                        nc.scalar.copy(dst, ps)

                    # v for this quarter's key tiles into [t, d] layout
                    _mark(nc, f"b{b}q{tq}:vtr")
                    for u in range(KTQ):
                        kt = tq * KTQ + u
                        vp = ytr_pp.tile([P, HD], F32, tag="ytr")
                        nc.tensor.transpose(vp, vTq[:, u * P:(u + 1) * P],
                                            identity)
                        nc.vector.tensor_copy(
                            v_sb[:, kt * HD:(kt + 1) * HD], vp)

                # ---- causal attention for this query chunk ----
                qc = tq
                last_chunk = (b == B - 1) and (tq == NQC - 1)
                nkt = (qc + 1) * KTQ
                ngr = nkt // KGRP

                sums2 = sums_sb_pool.tile([(HPC - 1) * 32 + 1, QC], F32,
                                           tag="ssb")
                nc.gpsimd.memset(sums2, 1.0)
                ysums = []
                for h in range(HPC):
                  with nc.named_scope(f"attn{b}q{tq}h{h}"):
                    y_ps = y_pp.tile([P, QC], F32, tag="y")
                    s_ps = sums_pp.tile([1, QC], F32, tag="sums")
                    qrhs = qTq[:, h]

                    def s_mm(g):
                        st = st_pp.tile([P, KGRP, QC], F32, tag="st")
                        for u in range(KGRP):
                            kt = g * KGRP + u
                            nc.tensor.matmul(
                                st[:, u], lhsT=kT[:, kt * P:(kt + 1) * P],
                                rhs=qrhs, start=True, stop=True)
                        pt = pt_pool.tile([P, KGRP, QC], F32R, tag="pt")
                        nc.scalar.activation(
                            pt, st, mybir.ActivationFunctionType.Exp,
                            scale=inv_sqrt_hd)
                        if g == 2 * qc:
                            nc.vector.tensor_mul(pt, pt, masks[0])
                        elif g == 2 * qc + 1:
                            nc.vector.tensor_mul(pt, pt, masks[1])
                        return pt

                    # S/exp run one group ahead of PV/sums so the PE has
                    # score matmuls to chew on while ACT exps the previous
                    # group (st double-buffer bounds the lookahead at 1).
                    pts = {0: s_mm(0)}
                    for g in range(ngr):
                        _mark(nc, f"b{b}q{tq}:att{h}g{g}")
                        if g + 1 < ngr:
                            pts[g + 1] = s_mm(g + 1)
                        pt = pts.pop(g)
                        first, last = g == 0, g == ngr - 1
                        for u in range(KGRP):
                            kt = g * KGRP + u
                            prhs = pt[:, u]
                            nc.tensor.matmul(
                                y_ps, lhsT=v_sb[:, kt * HD:(kt + 1) * HD],
                                rhs=prhs,
                                start=(first and u == 0),
                                stop=(last and u == KGRP - 1))
                            nc.tensor.matmul(
                                s_ps, lhsT=ones_col, rhs=prhs,
                                start=(first and u == 0),
                                stop=(last and u == KGRP - 1))
                    # Free the accumulation banks now (DVE-only), but defer
                    # the tail's PE work (broadcast matmul + transposes) so
                    # it queues behind the next chunk's matmuls — by then the
                    # reciprocal is long done and the PE never stalls on it.
                    _mark(nc, f"b{b}q{tq}:tail{h}")
                    nc.vector.tensor_copy(sums2[h * 32:h * 32 + 1, :], s_ps)
                    ysum = ysum_pool.tile([P, QC], F32, tag="ysum")
                    nc.vector.tensor_copy(ysum, y_ps)
                    ysums.append(ysum)
                pending_tails.append((b, qc, ysums, sums2))
                while len(pending_tails) > (0 if last_chunk else 1):
                    emit_tail(*pending_tails.pop(0), terminal=last_chunk)

    nc.compile()
    return nc


_cache = {}


def _get_nc(B, T, C, HPC):
    key = (B, T, C, HPC)
    if key not in _cache:
        _cache[key] = build_nc(B, T, C, HPC)
    return _cache[key]


def prepare_in_maps(x, w_kv, w_q):
    x = np.asarray(x)
    n_head = 16
    hpc = n_head // N_CORES
    xT = np.ascontiguousarray(x.transpose(0, 2, 1)).astype(np.float32)
    wkv_t = np.ascontiguousarray(np.asarray(w_kv, dtype=np.float32).T)
    in_maps = []
    for i in range(N_CORES):
        wq_sh = np.ascontiguousarray(
            np.asarray(w_q, dtype=np.float32)[i * hpc * HD:(i + 1) * hpc * HD].T)
        in_maps.append({"xT": xT, "wq_t": wq_sh, "wkv_t": wkv_t})
    return in_maps


def gather_output(results):
    return np.concatenate([results[i]["y"] for i in range(N_CORES)], axis=-1)


def kernel(x, w_kv, w_q):
    x = np.asarray(x)
    B, T, C = x.shape
    nc = _get_nc(B, T, C, 16 // N_CORES)
    in_maps = prepare_in_maps(x, w_kv, w_q)
    res = run_bass_kernel_spmd(nc, in_maps, list(range(N_CORES)))
    return gather_output(res.results)



# revision 17
# speedup vs baseline: 1.2784x; 1.2784x over previous
"""Causal MQA kernel for Trainium2, SPMD over 8 NeuronCores.

Sharding: 2 batches x 4 head-groups = 8 cores. Core c handles batch c//4 and
query heads [4*(c%4), 4*(c%4)+4) (MQA TP layout: the single shared KV head is
computed per-core for its batch only). Host concatenates output slices.

Per-core algorithm (T=2048 processed in 4 query chunks of QC=512):
  - chunk 0 runs in bf16 end-to-end: its rows have few causal keys, so
    softmax can't average away quantization noise; bf16 keeps the worst-case
    (row 0 copies v[0]) at ~0.1% error.
  - chunks 1-3 run in fp8e4 with DoubleRow matmuls (2 contraction tiles per
    instruction, 0.5 cyc/row):
      * projections contract C=2048 as 8 pairs of 128-chunks,
      * scores contract d=128 folded onto 64 partitions as [64, 2, .],
      * PV and the softmax-denominator (ones) matmul pair key tiles.
    fp8 weights are prescaled x16 to dodge subnormals; exp absorbs the 1/256.
  - diagonal key tiles compute/exp only the causal column range; stale pt
    regions are zeroed on GpSimd and the 128x128 diagonal triangles masked on
    DVE. Denominators for all 4 heads accumulate in one PSUM bank at
    partitions 32h (tile_position trick); reciprocal runs on [97, 512].
  - projections of chunk t+1 are interleaved into the (ACT-exp-bound)
    attention of chunk t so the PE stays busy.
"""

import math
from contextlib import ExitStack

import numpy as np

import concourse.bass as bass
import concourse.mybir as mybir
import concourse.tile as tile
from concourse import bacc
from concourse.bass_utils import run_bass_kernel_spmd
from concourse.masks import make_identity

F32 = mybir.dt.float32
F32R = mybir.dt.float32r
BF16 = mybir.dt.bfloat16
F8 = mybir.dt.float8e4
DR = mybir.MatmulPerfMode.DoubleRow

P = 128
HD = 128
QC = 512          # query chunk width (one fp32 PSUM bank)
KGRP = 2          # key tiles per group / DoubleRow pair
N_CORES = 8
HPC = 4           # query heads per core
SW = 16.0         # fp8 weight prescale (avoids e4m3 subnormals)
NS2 = (HPC - 1) * 32 + 1  # sums packed at partitions 32h

PHASE_MARKS = []


def _mark(nc, name):
    n = int(nc.get_next_instruction_name().split("-")[-1])
    PHASE_MARKS.append((n, name))


def _dummy_gen():
    while True:
        yield


def build_nc(T, C):
    NQC = T // QC          # query chunks (4)
    NCC = C // P           # contraction chunks (16)
    NCCP = NCC // 2        # contraction pairs (8)
    KTQ = QC // P          # key tiles per chunk (4)
    NKT = T // P           # total key tiles (16)
    DOUT = 2 * HD + HPC * HD  # 768 projection outputs: k, v, q0..q3
    inv_sqrt_hd = 1.0 / math.sqrt(HD)
    s_scale16 = inv_sqrt_hd
    s_scale8 = inv_sqrt_hd / (SW * SW)

    nc = bacc.Bacc("TRN2", target_bir_lowering=False, debug=False,
                   num_devices=N_CORES)
    xt16_d = nc.dram_tensor("xt16", [C, 2 * QC], BF16,
                            kind="ExternalInput").ap()
    x8_d = nc.dram_tensor("x8", [C, T], F8, kind="ExternalInput").ap()
    w16_d = nc.dram_tensor("w16", [C, DOUT], BF16, kind="ExternalInput").ap()
    w8_d = nc.dram_tensor("w8", [C, DOUT], F8, kind="ExternalInput").ap()
    y = nc.dram_tensor("y", [T, HPC * HD], F32, kind="ExternalOutput").ap()

    with tile.TileContext(nc) as tc, ExitStack() as ctx, \
            nc.allow_low_precision(reason="bf16 chunk-0 + fp8 DoubleRow attention; fp32 PSUM accumulation"):
        consts = ctx.enter_context(tc.tile_pool(name="consts", bufs=1))
        ident32 = consts.tile([P, P], F32)
        make_identity(nc, ident32)
        identR = consts.tile([P, P], F32R, tag="identR")
        nc.vector.tensor_copy(identR, ident32)
        ident16 = consts.tile([P, P], BF16, tag="ident16")
        nc.vector.tensor_copy(ident16, ident32)
        ident8 = consts.tile([P, P], F8, tag="ident8")
        nc.vector.tensor_copy(ident8, ident32)

        ones32 = consts.tile([P, 2], F32, tag="ones32")
        nc.gpsimd.memset(ones32, 1.0)
        ones16 = consts.tile([P, 1], BF16, tag="ones16")
        nc.vector.tensor_copy(ones16, ones32[:, 0:1])
        # DoubleRow ldweights rejects stationary free size (2,1); use a
        # 32-wide ones block so each head writes 32 identical sum rows
        ones8 = consts.tile([P, 2, 32], F8, tag="ones8")
        nc.gpsimd.memset(ones8.rearrange("p u o -> p (u o)"), 1.0)

        # tri[k, q] = 1 iff q >= k (within a 128x128 diagonal block)
        tri32 = consts.tile([P, P], F32, tag="tri32")
        nc.gpsimd.memset(tri32, 1.0)
        nc.gpsimd.affine_select(
            out=tri32, in_=tri32,
            pattern=[[1, P]],
            compare_op=mybir.AluOpType.is_ge,
            fill=0.0, base=0, channel_multiplier=-1)
        tri16 = consts.tile([P, P], BF16, tag="tri16")
        nc.vector.tensor_copy(tri16, tri32)
        tri8 = consts.tile([P, P], F8, tag="tri8")
        nc.vector.tensor_copy(tri8, tri32)

        # weights
        w16_sb = consts.tile([P, NCC, DOUT], BF16, tag="w16")
        w16_r = w16_d.rearrange("(cc p) d -> p cc d", p=P)
        for c0 in range(0, NCC, 4):
            nc.sync.dma_start(out=w16_sb[:, c0:c0 + 4], in_=w16_r[:, c0:c0 + 4])
        w8_sb = consts.tile([P, NCCP, 2, DOUT], F8, tag="w8")
        w8_r = w8_d.rearrange("(ccp u p) d -> p ccp u d", p=P, u=2)
        for c0 in range(0, NCCP, 4):
            nc.sync.dma_start(out=w8_sb[:, c0:c0 + 4], in_=w8_r[:, c0:c0 + 4])

        xt16_sb = consts.tile([P, NCC, 2 * QC], BF16, tag="xt16")
        xt16_r = xt16_d.rearrange("(cc p) t -> p cc t", p=P)

        x8_pool = ctx.enter_context(tc.tile_pool(name="x8", bufs=2))
        x8_r = x8_d.rearrange("(ccp u p) t -> p ccp u t", p=P, u=2)

        # q/k stored x16 in bf16 (scores run bf16: fp8 score noise was
        # the dominant error term); exp absorbs the 1/256
        kT16f = consts.tile([P, T], BF16, tag="kT16f")
        qT_pool = ctx.enter_context(tc.tile_pool(name="qT", bufs=2))
        v16 = consts.tile([P, KTQ * HD], BF16, tag="v16")  # chunk-0 v [k, d]
        v8 = consts.tile([P, NKT, HD], F8, tag="v8")       # all v [k, d]
        vt_pool = ctx.enter_context(tc.tile_pool(name="vt", bufs=2))
        pt16_pool = ctx.enter_context(tc.tile_pool(name="pt16", bufs=2))
        pt8_pool = ctx.enter_context(tc.tile_pool(name="pt8", bufs=4))
        sums_sb_pool = ctx.enter_context(tc.tile_pool(name="ssb", bufs=2))
        recip_pool = ctx.enter_context(tc.tile_pool(name="recip", bufs=2))
        ysum_pool = ctx.enter_context(tc.tile_pool(name="ysum", bufs=8))
        yout_pool = ctx.enter_context(tc.tile_pool(name="yout", bufs=3))

        # PSUM budget (8 banks): st 2x2 + y 2 + sums 1 + ytr 1
        st_pp = ctx.enter_context(tc.tile_pool(name="st_pp", bufs=2, space="PSUM"))
        y_pp = ctx.enter_context(tc.tile_pool(name="y_pp", bufs=2, space="PSUM"))
        sums_pp = ctx.enter_context(tc.tile_pool(name="sums_pp", bufs=1, space="PSUM"))
        ytr_pp = ctx.enter_context(tc.tile_pool(name="ytr_pp", bufs=1, space="PSUM"))

        x8_tiles = {}
        qTs = {}
        for tq in range(NQC):
            qT_t = qT_pool.tile([P, HPC, QC], BF16, tag="qT")
            qTs[tq] = qT_t

        # ---------------- projections ----------------

        def proj_outputs(tq):
            """Generator emitting the 6 projection outputs of chunk tq in
            4 steps (k | v | q0,q1 | q2,q3)."""
            tslc = slice(tq * QC, (tq + 1) * QC)

            def finish(kind, ps, h):
                if kind == "k":
                    if tq <= 1:
                        nc.vector.tensor_scalar_mul(kT16f[:, tslc], ps, SW)
                    else:
                        nc.vector.tensor_copy(kT16f[:, tslc], ps)
                elif kind == "v":
                    if tq == 0:
                        vt = vt_pool.tile([P, QC], BF16, tag="vt16")
                        nc.scalar.copy(vt, ps)
                        for u in range(KTQ):
                            tp = ytr_pp.tile([P, HD], BF16, tag="ytr")
                            nc.tensor.transpose(
                                tp, vt[:, u * P:(u + 1) * P], ident16)
                            nc.vector.tensor_copy(v16[:, u * HD:(u + 1) * HD], tp)
                            nc.vector.tensor_copy(v8[:, u], tp)
                    else:
                        vt = vt_pool.tile([P, QC], BF16, tag="vt16")
                        nc.scalar.activation(
                            vt, ps, mybir.ActivationFunctionType.Copy,
                            scale=1.0 / SW)
                        for u in range(KTQ):
                            kt = tq * KTQ + u
                            tp = ytr_pp.tile([P, HD], BF16, tag="ytr")
                            nc.tensor.transpose(
                                tp, vt[:, u * P:(u + 1) * P], ident16)
                            nc.vector.tensor_copy(v8[:, kt], tp)
                else:  # q head h
                    if tq <= 1:
                        nc.vector.tensor_scalar_mul(qTs[tq][:, h], ps, SW)
                    else:
                        nc.vector.tensor_copy(qTs[tq][:, h], ps)

            def emit(kind, off, h=None):
                ps = st_pp.tile([P, QC], F32, tag="st")
                if tq == 0 or (tq == 1 and kind != "v"):
                    for cc in range(NCC):
                        nc.tensor.matmul(
                            ps, lhsT=w16_sb[:, cc, off:off + P],
                            rhs=xt16_sb[:, cc, tq * QC:(tq + 1) * QC],
                            start=(cc == 0), stop=(cc == NCC - 1))
                else:
                    x8t = x8_tiles[tq]
                    for ccp in range(NCCP):
                        nc.tensor.matmul(
                            ps, lhsT=w8_sb[:, ccp, :, off:off + P],
                            rhs=x8t[:, ccp],
                            start=(ccp == 0), stop=(ccp == NCCP - 1),
                            perf_mode=DR)
                finish(kind, ps, h)

            with nc.named_scope(f"proj{tq}k"):
                _mark(nc, f"proj{tq}:k")
                emit("k", 0)
            yield
            with nc.named_scope(f"proj{tq}v"):
                _mark(nc, f"proj{tq}:v")
                emit("v", HD)
            yield
            for h in range(HPC):
                with nc.named_scope(f"proj{tq}q{h}"):
                    _mark(nc, f"proj{tq}:q{h}")
                    emit("q", 2 * HD + h * HD, h)
                if h % 2 == 1:
                    yield
            while True:
                yield

        # ---------------- attention ----------------

        def attn_chunk0_head(h, y_ps, sums_ps):
            """bf16 attention for chunk 0 head h: 4 diagonal key tiles,
            S/exp causally trimmed, stale pt regions zeroed, full-width
            PV/sums."""
            pt = pt16_pool.tile([P, KTQ, QC], BF16, tag="pt16")
            for pair in range(2):
                st = st_pp.tile([P, KGRP, QC], F32, tag="st")
                for u in range(2):
                    j = 2 * pair + u
                    q0 = j * P
                    nc.tensor.matmul(
                        st[:, u, q0:], lhsT=kT16f[:, j * P:(j + 1) * P],
                        rhs=qTs[0][:, h, q0:], start=True, stop=True)
                for u in range(2):
                    j = 2 * pair + u
                    q0 = j * P
                    nc.scalar.activation(
                        pt[:, j, q0:], st[:, u, q0:],
                        mybir.ActivationFunctionType.Exp, scale=s_scale8)
            for j in range(1, KTQ):
                nc.gpsimd.memset(pt[:, j, :j * P], 0.0)
            for j in range(KTQ):
                nc.vector.tensor_mul(
                    pt[:, j, j * P:(j + 1) * P],
                    pt[:, j, j * P:(j + 1) * P], tri16)
            for j in range(KTQ):
                nc.tensor.matmul(
                    y_ps, lhsT=v16[:, j * HD:(j + 1) * HD], rhs=pt[:, j],
                    start=(j == 0), stop=(j == KTQ - 1))
                nc.tensor.matmul(
                    sums_ps[0:1], lhsT=ones16, rhs=pt[:, j],
                    start=(j == 0), stop=(j == KTQ - 1))

        def attn_fp8_head(tq, h, y_ps, sums_ps, proj_gen):
            """fp8 DoubleRow attention for chunk tq>=1 head h."""
            qTc = qTs[tq]
            nkt = (tq + 1) * KTQ
            ngr_off = (nkt - KTQ) // KGRP
            D = nkt - KTQ

            def s_off(g):
                st = st_pp.tile([P, KGRP, QC], F32, tag="st")
                for u in range(2):
                    kt = KGRP * g + u
                    nc.tensor.matmul(
                        st[:, u], lhsT=kT16f[:, kt * P:(kt + 1) * P],
                        rhs=qTc[:, h], start=True, stop=True)
                pt = pt8_pool.tile([P, KGRP, QC], F8, tag="pt8")
                nc.scalar.activation(
                    pt, st, mybir.ActivationFunctionType.Exp, scale=s_scale8)
                return pt

            def s_diag(pair):
                st = st_pp.tile([P, KGRP, QC], F32, tag="st")
                for u in range(2):
                    j = 2 * pair + u
                    kt = D + j
                    nc.tensor.matmul(
                        st[:, u], lhsT=kT16f[:, kt * P:(kt + 1) * P],
                        rhs=qTc[:, h], start=True, stop=True)
                pt = pt8_pool.tile([P, KGRP, QC], F8, tag="pt8")
                for u in range(2):
                    j = 2 * pair + u
                    q0 = j * P
                    nc.scalar.activation(
                        pt[:, u, q0:], st[:, u, q0:],
                        mybir.ActivationFunctionType.Exp, scale=s_scale8)
                for u in range(2):
                    j = 2 * pair + u
                    if j > 0:
                        nc.gpsimd.memset(pt[:, u, :j * P], 0.0)
                    nc.vector.tensor_mul(
                        pt[:, u, j * P:(j + 1) * P],
                        pt[:, u, j * P:(j + 1) * P], tri8)
                return pt

            producers = [lambda g=g: s_off(g) for g in range(ngr_off)]
            producers += [lambda p=p: s_diag(p) for p in range(2)]
            ngr = len(producers)
            pts = {0: producers[0]()}
            for g in range(ngr):
                _mark(nc, f"q{tq}:att{h}g{g}")
                if g + 1 < ngr:
                    pts[g + 1] = producers[g + 1]()
                pt = pts.pop(g)
                nc.tensor.matmul(
                    y_ps, lhsT=v8[:, KGRP * g:KGRP * (g + 1)], rhs=pt,
                    start=(g == 0), stop=(g == ngr - 1), perf_mode=DR)
                nc.tensor.matmul(
                    sums_ps, lhsT=ones8, rhs=pt,
                    start=(g == 0), stop=(g == ngr - 1), perf_mode=DR)
                next(proj_gen)

        # ---------------- tail ----------------

        pending_tails = []

        def emit_tail(tq, ysums, sums2, terminal=False):
            # Transpose the packed sums to [q, head-cols] so the reciprocal
            # uses all DVE lanes, then fold normalization into per-partition
            # ACT scales on the PSUM->SBUF copies of the transposed output.
            with nc.named_scope(f"tail{tq}"):
                _mark(nc, f"tail{tq}")
                pp = st_pp if terminal else ytr_pp
                tg = "st" if terminal else "ytr"
                rt_ps = pp.tile([P, KTQ, NS2], F32, tag=tg)
                for qt in range(KTQ):
                    nc.tensor.transpose(
                        rt_ps[:, qt], sums2[:, qt * P:(qt + 1) * P],
                        ident32[0:NS2, 0:NS2])
                rt = recip_pool.tile([P, KTQ, NS2], F32, tag="recip")
                nc.vector.reciprocal(rt, rt_ps)
                for th in range(HPC):
                    ytr = pp.tile([P, QC], BF16, tag=tg)
                    ysr = ysums[th]
                    for qt in range(KTQ):
                        nc.tensor.transpose(
                            ytr[:, qt * P:(qt + 1) * P],
                            ysr[:, qt * P:(qt + 1) * P], ident16)
                    yo = yout_pool.tile([P, QC], F32, tag="yo")
                    ytf = ytr
                    for qt in range(KTQ):
                        nc.scalar.activation(
                            yo[:, qt * P:(qt + 1) * P],
                            ytf[:, qt * P:(qt + 1) * P],
                            mybir.ActivationFunctionType.Copy,
                            scale=rt[:, qt, 32 * th:32 * th + 1])
                    ydst = y.rearrange(
                        "(nq qt p) (h d) -> nq h p qt d",
                        qt=KTQ, p=P, h=HPC)[tq, th]
                    nc.sync.dma_start(
                        out=ydst,
                        in_=yo.rearrange("p (qt d) -> p qt d", qt=KTQ))

        # ---------------- main schedule ----------------

        def load_x8(tq):
            x8t = x8_pool.tile([P, NCCP, 2, QC], F8, tag="x8")
            tslc = slice(tq * QC, (tq + 1) * QC)
            for c0 in range(0, NCCP, 4):
                nc.sync.dma_start(out=x8t[:, c0:c0 + 4],
                                  in_=x8_r[:, c0:c0 + 4, :, tslc])
            x8_tiles[tq] = x8t

        # chunk-0 inputs + projections up-front
        for c0 in range(0, NCC, 4):
            nc.sync.dma_start(out=xt16_sb[:, c0:c0 + 4],
                              in_=xt16_r[:, c0:c0 + 4])
        load_x8(1)
        g0 = proj_outputs(0)
        for _ in range(4):
            next(g0)

        for tq in range(NQC):
            _mark(nc, f"chunk{tq}")
            if tq + 2 < NQC:
                load_x8(tq + 2)
            proj_gen = proj_outputs(tq + 1) if tq + 1 < NQC else _dummy_gen()
            sums2 = sums_sb_pool.tile([NS2, QC], F32, tag="ssb")
            ysums = []
            for h in range(HPC):
                with nc.named_scope(f"attn{tq}h{h}"):
                    y_ps = y_pp.tile([P, QC], F32, tag="y")
                    sums_ps = sums_pp.tile([32, QC], F32, tag="sums")
                    if tq == 0:
                        attn_chunk0_head(h, y_ps, sums_ps)
                        next(proj_gen)
                    else:
                        attn_fp8_head(tq, h, y_ps, sums_ps, proj_gen)
                    nc.vector.tensor_copy(sums2[32 * h:32 * h + 1],
                                          sums_ps[0:1])
                    ysum = ysum_pool.tile([P, QC], BF16, tag="ysum")
                    nc.vector.tensor_copy(ysum, y_ps)
                    ysums.append(ysum)
            # drain remaining proj work for the next chunk
            for _ in range(6):
                next(proj_gen)
            last_chunk = tq == NQC - 1
            pending_tails.append((tq, ysums, sums2))
            while len(pending_tails) > (0 if last_chunk else 1):
                emit_tail(*pending_tails.pop(0), terminal=last_chunk)

    nc.compile()
    return nc


_cache = {}


def _get_nc(T, C):
    key = (T, C)
    if key not in _cache:
        _cache[key] = build_nc(T, C)
    return _cache[key]


def prepare_in_maps(x, w_kv, w_q):
    import ml_dtypes
    bf16 = ml_dtypes.bfloat16
    f8 = ml_dtypes.float8_e4m3
    x = np.asarray(x, dtype=np.float32)
    w_kv = np.asarray(w_kv, dtype=np.float32)
    w_q = np.asarray(w_q, dtype=np.float32)
    B = x.shape[0]
    xTs = [np.ascontiguousarray(x[b].T) for b in range(B)]
    xt16s = [np.ascontiguousarray(xT[:, :2 * QC]).astype(bf16)
             for xT in xTs]
    x8s = [xT.astype(f8) for xT in xTs]
    n_hg = N_CORES // B
    w16s, w8s = [], []
    for hg in range(n_hg):
        W = np.concatenate(
            [w_kv, w_q[hg * HPC * HD:(hg + 1) * HPC * HD]], axis=0)
        WT = np.ascontiguousarray(W.T)
        w16s.append(WT.astype(bf16))
        w8s.append((WT * SW).astype(f8))
    in_maps = []
    for core in range(N_CORES):
        b, hg = core // n_hg, core % n_hg
        in_maps.append({"xt16": xt16s[b], "x8": x8s[b],
                        "w16": w16s[hg], "w8": w8s[hg]})
    return in_maps


def gather_output(results):
    n_hg = N_CORES // 2
    outs = []
    for b in range(2):
        outs.append(np.concatenate(
            [results[b * n_hg + hg]["y"] for hg in range(n_hg)], axis=-1))
    return np.stack(outs, axis=0)


def kernel(x, w_kv, w_q):
    x = np.asarray(x)
    B, T, C = x.shape
    nc = _get_nc(T, C)
    in_maps = prepare_in_maps(x, w_kv, w_q)
    res = run_bass_kernel_spmd(nc, in_maps, list(range(N_CORES)))
    return gather_output(res.results)


# revision 20
# speedup vs baseline: 1.3473x; 1.0539x over previous
"""Causal MQA kernel for Trainium2, SPMD over 8 NeuronCores.

Sharding: 2 batches x 4 head-groups = 8 cores. Core c handles batch c//4 and
query heads [4*(c%4), 4*(c%4)+4) (MQA TP layout: the single shared KV head is
computed per-core for its batch only). Host concatenates output slices.

Per-core algorithm (T=2048 processed in 4 query chunks of QC=512):
  - chunk 0 runs in bf16 end-to-end, and chunk 1's q/k projections are also
    bf16: short causal rows can't average away quantization noise, and fp8
    q/k projection noise was measured as the dominant error term there.
  - everything else runs fp8e4 with DoubleRow matmuls (2 contraction tiles
    per instruction, 0.5 cyc/row): projections contract C=2048 as 8 pairs of
    128-chunks; PV and the softmax-denominator (ones) matmul pair key tiles.
    Scores always run bf16 (q/k stored x16 bf16; exp absorbs the 1/256).
  - diagonal key tiles exp only the causal column range; stale pt regions
    are zeroed and the 128x128 diagonal triangles masked on GpSimd/DVE.
  - projections of chunk t+1 are emitted in 4-matmul quanta between the
    (ACT-exp-bound) attention groups of chunk t, from a dedicated PSUM slot
    so the attention score pipeline keeps its 2-group lookahead.
  - DMA order: chunk-0 k/v weights + chunk-0 x first so the first
    projection starts immediately; fp8 tensors stream later.
"""

import math
from contextlib import ExitStack

import numpy as np

import concourse.bass as bass
import concourse.mybir as mybir
import concourse.tile as tile
from concourse import bacc
from concourse.bass_utils import run_bass_kernel_spmd
from concourse.masks import make_identity

F32 = mybir.dt.float32
F32R = mybir.dt.float32r
BF16 = mybir.dt.bfloat16
F8 = mybir.dt.float8e4
DR = mybir.MatmulPerfMode.DoubleRow

P = 128
HD = 128
QC = 512          # query chunk width (one fp32 PSUM bank)
KGRP = 2          # key tiles per group / DoubleRow pair
N_CORES = 8
HPC = 4           # query heads per core
SW = 16.0         # fp8 weight prescale (avoids e4m3 subnormals)
NS2 = (HPC - 1) * 32 + 1  # sums packed at partitions 32h in SBUF

PHASE_MARKS = []


def _mark(nc, name):
    n = int(nc.get_next_instruction_name().split("-")[-1])
    PHASE_MARKS.append((n, name))


def _dummy_gen():
    while True:
        yield


def build_nc(T, C):
    NQC = T // QC          # query chunks (4)
    NCC = C // P           # contraction chunks (16)
    NCCP = NCC // 2        # contraction pairs (8)
    KTQ = QC // P          # key tiles per chunk (4)
    NKT = T // P           # total key tiles (16)
    DOUT = 2 * HD + HPC * HD  # 768 projection outputs: k, v, q0..q3
    inv_sqrt_hd = 1.0 / math.sqrt(HD)
    s_scale8 = inv_sqrt_hd / (SW * SW)

    nc = bacc.Bacc("TRN2", target_bir_lowering=False, debug=False,
                   num_devices=N_CORES)
    xt16_d = nc.dram_tensor("xt16", [C, 2 * QC], BF16,
                            kind="ExternalInput").ap()
    x8_d = nc.dram_tensor("x8", [C, T], F8, kind="ExternalInput").ap()
    w16_d = nc.dram_tensor("w16", [C, DOUT], BF16, kind="ExternalInput").ap()
    w8_d = nc.dram_tensor("w8", [C, DOUT], F8, kind="ExternalInput").ap()
    y = nc.dram_tensor("y", [T, HPC * HD], F32, kind="ExternalOutput").ap()

    with tile.TileContext(nc) as tc, ExitStack() as ctx, \
            nc.allow_low_precision(reason="bf16 scores/chunk-0 + fp8 DoubleRow PV/projections; fp32 PSUM accumulation"):
        consts = ctx.enter_context(tc.tile_pool(name="consts", bufs=1))
        ident32 = consts.tile([P, P], F32)
        make_identity(nc, ident32)
        ident16 = consts.tile([P, P], BF16, tag="ident16")
        nc.vector.tensor_copy(ident16, ident32)

        ones32 = consts.tile([P, 2], F32, tag="ones32")
        nc.gpsimd.memset(ones32, 1.0)
        ones16 = consts.tile([P, 1], BF16, tag="ones16")
        nc.vector.tensor_copy(ones16, ones32[:, 0:1])
        ones8 = consts.tile([P, 2, P], F8, tag="ones8")
        nc.gpsimd.memset(ones8.rearrange("p u o -> p (u o)"), 1.0)

        # tri[k, q] = 1 iff q >= k (within a 128x128 diagonal block)
        tri32 = consts.tile([P, P], F32, tag="tri32")
        nc.gpsimd.memset(tri32, 1.0)
        nc.gpsimd.affine_select(
            out=tri32, in_=tri32,
            pattern=[[1, P]],
            compare_op=mybir.AluOpType.is_ge,
            fill=0.0, base=0, channel_multiplier=-1)
        tri16 = consts.tile([P, P], BF16, tag="tri16")
        nc.vector.tensor_copy(tri16, tri32)
        tri8 = consts.tile([P, P], F8, tag="tri8")
        nc.vector.tensor_copy(tri8, tri32)

        # weights / inputs (DMA emitted in priority order below)
        w16_sb = consts.tile([P, NCC, DOUT], BF16, tag="w16")
        w16_r = w16_d.rearrange("(cc p) d -> p cc d", p=P)
        w8_sb = consts.tile([P, NCCP, 2, DOUT], F8, tag="w8")
        w8_r = w8_d.rearrange("(ccp u p) d -> p ccp u d", p=P, u=2)
        xt16_sb = consts.tile([P, NCC, 2 * QC], BF16, tag="xt16")
        xt16_r = xt16_d.rearrange("(cc p) t -> p cc t", p=P)
        x8_pool = ctx.enter_context(tc.tile_pool(name="x8", bufs=2))
        x8_r = x8_d.rearrange("(ccp u p) t -> p ccp u t", p=P, u=2)

        # chunk-0 k/v weight columns + chunk-0 x first: the first projection
        # can start after ~1.5 MB instead of the full 10 MB input stream.
        for c0 in range(0, NCC, 4):
            nc.sync.dma_start(out=w16_sb[:, c0:c0 + 4, 0:2 * HD],
                              in_=w16_r[:, c0:c0 + 4, 0:2 * HD])
            nc.sync.dma_start(out=xt16_sb[:, c0:c0 + 4, 0:QC],
                              in_=xt16_r[:, c0:c0 + 4, 0:QC])
        for c0 in range(0, NCC, 4):
            nc.sync.dma_start(out=w16_sb[:, c0:c0 + 4, 2 * HD:],
                              in_=w16_r[:, c0:c0 + 4, 2 * HD:])
        for c0 in range(0, NCC, 4):
            nc.sync.dma_start(out=xt16_sb[:, c0:c0 + 4, QC:],
                              in_=xt16_r[:, c0:c0 + 4, QC:])

        # q/k stored x16 in bf16; scores run bf16
        kT16f = consts.tile([P, T], BF16, tag="kT16f")
        qT_pool = ctx.enter_context(tc.tile_pool(name="qT", bufs=2))
        v16 = consts.tile([P, KTQ * HD], BF16, tag="v16")  # chunk-0 v [k, d]
        v8 = consts.tile([P, NKT, HD], F8, tag="v8")       # all v [k, d]
        vt_pool = ctx.enter_context(tc.tile_pool(name="vt", bufs=2))
        pt16_pool = ctx.enter_context(tc.tile_pool(name="pt16", bufs=2))
        pt8_pool = ctx.enter_context(tc.tile_pool(name="pt8", bufs=6))
        sums_sb_pool = ctx.enter_context(tc.tile_pool(name="ssb", bufs=2))
        recip_pool = ctx.enter_context(tc.tile_pool(name="recip", bufs=2))
        ysum_pool = ctx.enter_context(tc.tile_pool(name="ysum", bufs=8))
        yout_pool = ctx.enter_context(tc.tile_pool(name="yout", bufs=3))

        # PSUM budget (8 banks): st 2x2 + y 2 + sums 1 + ytr/proj 1
        st_pp = ctx.enter_context(tc.tile_pool(name="st_pp", bufs=2, space="PSUM"))
        y_pp = ctx.enter_context(tc.tile_pool(name="y_pp", bufs=2, space="PSUM"))
        sums_pp = ctx.enter_context(tc.tile_pool(name="sums_pp", bufs=1, space="PSUM"))
        ytr_pp = ctx.enter_context(tc.tile_pool(name="ytr_pp", bufs=1, space="PSUM"))

        x8_tiles = {}
        qTs = {}
        for tq in range(NQC):
            qT_t = qT_pool.tile([P, HPC, QC], BF16, tag="qT")
            qTs[tq] = qT_t

        def load_x8(tq):
            x8t = x8_pool.tile([P, NCCP, 2, QC], F8, tag="x8")
            tslc = slice(tq * QC, (tq + 1) * QC)
            for c0 in range(0, NCCP, 4):
                nc.sync.dma_start(out=x8t[:, c0:c0 + 4],
                                  in_=x8_r[:, c0:c0 + 4, :, tslc])
            x8_tiles[tq] = x8t

        # ---------------- projections ----------------

        def proj_outputs(tq):
            """Generator emitting the 6 projection outputs of chunk tq,
            yielding every ~4 matmuls so attention groups can interleave.
            Uses the ytr PSUM slot, keeping st_pp free for the score
            pipeline."""
            tslc = slice(tq * QC, (tq + 1) * QC)

            def finish(kind, ps, h):
                if kind == "k":
                    if tq <= 1:
                        nc.vector.tensor_scalar_mul(kT16f[:, tslc], ps, SW)
                    else:
                        nc.vector.tensor_copy(kT16f[:, tslc], ps)
                elif kind == "v":
                    if tq == 0:
                        vt = vt_pool.tile([P, QC], BF16, tag="vt16")
                        nc.scalar.copy(vt, ps)
                    else:
                        vt = vt_pool.tile([P, QC], BF16, tag="vt16")
                        nc.scalar.activation(
                            vt, ps, mybir.ActivationFunctionType.Copy,
                            scale=1.0 / SW)
                    for u in range(KTQ):
                        kt = tq * KTQ + u
                        tp = ytr_pp.tile([P, HD], BF16, tag="ytr")
                        nc.tensor.transpose(
                            tp, vt[:, u * P:(u + 1) * P], ident16)
                        if tq == 0:
                            nc.vector.tensor_copy(
                                v16[:, u * HD:(u + 1) * HD], tp)
                        nc.vector.tensor_copy(v8[:, kt], tp)
                else:  # q head h
                    if tq <= 1:
                        nc.vector.tensor_scalar_mul(qTs[tq][:, h], ps, SW)
                    else:
                        nc.vector.tensor_copy(qTs[tq][:, h], ps)

            def emit(kind, off, h=None):
                ps = ytr_pp.tile([P, QC], F32, tag="ytr")
                if tq == 0 or (tq == 1 and kind != "v"):
                    for cc in range(NCC):
                        nc.tensor.matmul(
                            ps, lhsT=w16_sb[:, cc, off:off + P],
                            rhs=xt16_sb[:, cc, tq * QC:(tq + 1) * QC],
                            start=(cc == 0), stop=(cc == NCC - 1))
                        if cc % 4 == 3 and cc != NCC - 1:
                            yield
                else:
                    x8t = x8_tiles[tq]
                    for ccp in range(NCCP):
                        nc.tensor.matmul(
                            ps, lhsT=w8_sb[:, ccp, :, off:off + P],
                            rhs=x8t[:, ccp],
                            start=(ccp == 0), stop=(ccp == NCCP - 1),
                            perf_mode=DR)
                        if ccp == 3:
                            yield
                finish(kind, ps, h)
                yield

            _mark(nc, f"proj{tq}:k")
            yield from emit("k", 0)
            _mark(nc, f"proj{tq}:v")
            yield from emit("v", HD)
            for h in range(HPC):
                _mark(nc, f"proj{tq}:q{h}")
                yield from emit("q", 2 * HD + h * HD, h)
            while True:
                yield

        # ---------------- attention ----------------

        def attn_chunk0_head(h, y_ps, sums_ps, proj_gen):
            """bf16 attention for chunk 0 head h: 4 diagonal key tiles,
            S/exp causally trimmed, stale pt regions zeroed, full-width
            PV/sums."""
            pt = pt16_pool.tile([P, KTQ, QC], BF16, tag="pt16")
            for pair in range(2):
                st = st_pp.tile([P, KGRP, QC], F32, tag="st")
                for u in range(2):
                    j = 2 * pair + u
                    q0 = j * P
                    nc.tensor.matmul(
                        st[:, u, q0:], lhsT=kT16f[:, j * P:(j + 1) * P],
                        rhs=qTs[0][:, h, q0:], start=True, stop=True)
                for u in range(2):
                    j = 2 * pair + u
                    q0 = j * P
                    nc.scalar.activation(
                        pt[:, j, q0:], st[:, u, q0:],
                        mybir.ActivationFunctionType.Exp, scale=s_scale8)
                next(proj_gen)
            for j in range(1, KTQ):
                nc.gpsimd.memset(pt[:, j, :j * P], 0.0)
            for j in range(KTQ):
                nc.vector.tensor_mul(
                    pt[:, j, j * P:(j + 1) * P],
                    pt[:, j, j * P:(j + 1) * P], tri16)
            for j in range(KTQ):
                nc.tensor.matmul(
                    y_ps, lhsT=v16[:, j * HD:(j + 1) * HD], rhs=pt[:, j],
                    start=(j == 0), stop=(j == KTQ - 1))
                nc.tensor.matmul(
                    sums_ps[0:1], lhsT=ones16, rhs=pt[:, j],
                    start=(j == 0), stop=(j == KTQ - 1))
                next(proj_gen)

        def attn_fp8_head(tq, h, y_ps, sums_ps, proj_gen):
            """bf16-score / fp8-DoubleRow-PV attention for chunk tq>=1."""
            qTc = qTs[tq]
            nkt = (tq + 1) * KTQ
            ngr_off = (nkt - KTQ) // KGRP
            D = nkt - KTQ

            def s_off(g):
                st = st_pp.tile([P, KGRP, QC], F32, tag="st")
                for u in range(2):
                    kt = KGRP * g + u
                    nc.tensor.matmul(
                        st[:, u], lhsT=kT16f[:, kt * P:(kt + 1) * P],
                        rhs=qTc[:, h], start=True, stop=True)
                pt = pt8_pool.tile([P, KGRP, QC], F8, tag="pt8")
                nc.scalar.activation(
                    pt, st, mybir.ActivationFunctionType.Exp, scale=s_scale8)
                return pt

            def s_diag(pair):
                st = st_pp.tile([P, KGRP, QC], F32, tag="st")
                for u in range(2):
                    j = 2 * pair + u
                    kt = D + j
                    nc.tensor.matmul(
                        st[:, u], lhsT=kT16f[:, kt * P:(kt + 1) * P],
                        rhs=qTc[:, h], start=True, stop=True)
                pt = pt8_pool.tile([P, KGRP, QC], F8, tag="pt8")
                for u in range(2):
                    j = 2 * pair + u
                    q0 = j * P
                    nc.scalar.activation(
                        pt[:, u, q0:], st[:, u, q0:],
                        mybir.ActivationFunctionType.Exp, scale=s_scale8)
                for u in range(2):
                    j = 2 * pair + u
                    if j > 0:
                        nc.gpsimd.memset(pt[:, u, :j * P], 0.0)
                    nc.gpsimd.tensor_mul(
                        pt[:, u, j * P:(j + 1) * P],
                        pt[:, u, j * P:(j + 1) * P], tri8)
                return pt

            producers = [lambda g=g: s_off(g) for g in range(ngr_off)]
            producers += [lambda p=p: s_diag(p) for p in range(2)]
            ngr = len(producers)
            pts = {0: producers[0]()}
            for g in range(ngr):
                _mark(nc, f"q{tq}:att{h}g{g}")
                if g + 1 < ngr:
                    pts[g + 1] = producers[g + 1]()
                pt = pts.pop(g)
                nc.tensor.matmul(
                    y_ps, lhsT=v8[:, KGRP * g:KGRP * (g + 1)], rhs=pt,
                    start=(g == 0), stop=(g == ngr - 1), perf_mode=DR)
                nc.tensor.matmul(
                    sums_ps, lhsT=ones8, rhs=pt,
                    start=(g == 0), stop=(g == ngr - 1), perf_mode=DR)
                next(proj_gen)

        # ---------------- tail ----------------

        pending_tails = []

        def emit_tail(tq, ysums, sums2, terminal=False):
            # Transpose the packed sums to [q, head-cols] so the reciprocal
            # uses all DVE lanes, then fold normalization into per-partition
            # scales on the PSUM->SBUF copies of the transposed output
            # (heads 0-1 on ACT, heads 2-3 on DVE).
            with nc.named_scope(f"tail{tq}"):
                _mark(nc, f"tail{tq}")
                pp = st_pp if terminal else ytr_pp
                tg = "st" if terminal else "ytr"
                rt_ps = pp.tile([P, KTQ, NS2], F32, tag=tg)
                for qt in range(KTQ):
                    nc.tensor.transpose(
                        rt_ps[:, qt], sums2[:, qt * P:(qt + 1) * P],
                        ident32[0:NS2, 0:NS2])
                rt = recip_pool.tile([P, KTQ, NS2], F32, tag="recip")
                nc.vector.reciprocal(rt, rt_ps)
                for th in range(HPC):
                    ytr = pp.tile([P, QC], BF16, tag=tg)
                    ysr = ysums[th]
                    for qt in range(KTQ):
                        nc.tensor.transpose(
                            ytr[:, qt * P:(qt + 1) * P],
                            ysr[:, qt * P:(qt + 1) * P], ident16)
                    yo = yout_pool.tile([P, QC], F32, tag="yo")
                    for qt in range(KTQ):
                        scale = rt[:, qt, 32 * th:32 * th + 1]
                        if th < 2:
                            nc.scalar.activation(
                                yo[:, qt * P:(qt + 1) * P],
                                ytr[:, qt * P:(qt + 1) * P],
                                mybir.ActivationFunctionType.Copy,
                                scale=scale)
                        else:
                            nc.vector.tensor_scalar_mul(
                                yo[:, qt * P:(qt + 1) * P],
                                ytr[:, qt * P:(qt + 1) * P], scale)
                    ydst = y.rearrange(
                        "(nq qt p) (h d) -> nq h p qt d",
                        qt=KTQ, p=P, h=HPC)[tq, th]
                    nc.sync.dma_start(
                        out=ydst,
                        in_=yo.rearrange("p (qt d) -> p qt d", qt=KTQ))

        # ---------------- main schedule ----------------

        load_x8(1)
        for c0 in range(0, NCCP, 4):
            nc.sync.dma_start(out=w8_sb[:, c0:c0 + 4], in_=w8_r[:, c0:c0 + 4])
        g0 = proj_outputs(0)
        for _ in range(30):
            next(g0)

        for tq in range(NQC):
            _mark(nc, f"chunk{tq}")
            if tq + 2 < NQC:
                load_x8(tq + 2)
            proj_gen = proj_outputs(tq + 1) if tq + 1 < NQC else _dummy_gen()
            sums2 = sums_sb_pool.tile([NS2, QC], F32, tag="ssb")
            ysums = []
            for h in range(HPC):
                with nc.named_scope(f"attn{tq}h{h}"):
                    y_ps = y_pp.tile([P, QC], F32, tag="y")
                    sums_ps = sums_pp.tile([P, QC], F32, tag="sums")
                    if tq == 0:
                        attn_chunk0_head(h, y_ps, sums_ps, proj_gen)
                    else:
                        attn_fp8_head(tq, h, y_ps, sums_ps, proj_gen)
                    nc.vector.tensor_copy(sums2[32 * h:32 * h + 1],
                                          sums_ps[0:1])
                    ysum = ysum_pool.tile([P, QC], BF16, tag="ysum")
                    nc.vector.tensor_copy(ysum, y_ps)
                    ysums.append(ysum)
            # drain remaining proj work for the next chunk
            for _ in range(40):
                next(proj_gen)
            last_chunk = tq == NQC - 1
            pending_tails.append((tq, ysums, sums2))
            while len(pending_tails) > (0 if last_chunk else 1):
                emit_tail(*pending_tails.pop(0), terminal=last_chunk)

    nc.compile()
    return nc


_cache = {}


def _get_nc(T, C):
    key = (T, C)
    if key not in _cache:
        _cache[key] = build_nc(T, C)
    return _cache[key]


def prepare_in_maps(x, w_kv, w_q):
    import ml_dtypes
    bf16 = ml_dtypes.bfloat16
    f8 = ml_dtypes.float8_e4m3
    x = np.asarray(x, dtype=np.float32)
    w_kv = np.asarray(w_kv, dtype=np.float32)
    w_q = np.asarray(w_q, dtype=np.float32)
    B = x.shape[0]
    xTs = [np.ascontiguousarray(x[b].T) for b in range(B)]
    xt16s = [np.ascontiguousarray(xT[:, :2 * QC]).astype(bf16)
             for xT in xTs]
    x8s = [xT.astype(f8) for xT in xTs]
    n_hg = N_CORES // B
    w16s, w8s = [], []
    for hg in range(n_hg):
        W = np.concatenate(
            [w_kv, w_q[hg * HPC * HD:(hg + 1) * HPC * HD]], axis=0)
        WT = np.ascontiguousarray(W.T)
        w16s.append(WT.astype(bf16))
        w8s.append((WT * SW).astype(f8))
    in_maps = []
    for core in range(N_CORES):
        b, hg = core // n_hg, core % n_hg
        in_maps.append({"xt16": xt16s[b], "x8": x8s[b],
                        "w16": w16s[hg], "w8": w8s[hg]})
    return in_maps


def gather_output(results):
    n_hg = N_CORES // 2
    outs = []
    for b in range(2):
        outs.append(np.concatenate(
            [results[b * n_hg + hg]["y"] for hg in range(n_hg)], axis=-1))
    return np.stack(outs, axis=0)


def kernel(x, w_kv, w_q):
    x = np.asarray(x)
    B, T, C = x.shape
    nc = _get_nc(T, C)
    in_maps = prepare_in_maps(x, w_kv, w_q)
    res = run_bass_kernel_spmd(nc, in_maps, list(range(N_CORES)))
    return gather_output(res.results)
